# revision 38
# baseline (speedup 1.0000x reference)
"""Causal self-attention (B=2, T=2048, D=1024, H=16) on 8 TRN2 NeuronCores.

Sharding: data-parallel over batch (cores 0-3 -> batch 0, cores 4-7 -> batch 1),
tensor-parallel over heads (4 heads / 256 output dims per core). Each core
computes q/k/v projections for its heads, causal flash-style attention, and a
partial output projection (contraction over its 256 dims of Wo). The host sums
the 4 partials per batch and adds bo.

All matmul operands are bf16 (fp32 PSUM accumulation). Inputs/outputs move as
bf16, halving HBM traffic vs fp32. Weights and trailing x blocks load via
single merged DMAs to keep the sync engine's DMA-issue serialization off the
critical path; the attention inner loop runs exp two k-tiles ahead of the
accumulating value matmuls so PE never waits on the scalar engine.
"""
import sys

sys.path.insert(0, '/opt/trn_rl_repo')

import numpy as np

import concourse.bass as bass  # noqa: F401  (import keeps bass registered)
import concourse.mybir as mybir
import concourse.tile as tile
from concourse import bacc
from concourse.bass_utils import run_bass_kernel_spmd

F32 = mybir.dt.float32
BF16 = mybir.dt.bfloat16
AF = mybir.ActivationFunctionType

B, T, D, H, HD = 2, 2048, 1024, 16, 64
NCORES = 8
E = 256          # output dims per core (4 heads x 64)
DM = 8           # d_model chunks of 128
TQ = 512
NTQ = T // TQ    # 4
TKT = 128
NTKT = T // TKT  # 16

_CACHE = {}


def _build(sim_safe=False):
    nc = bacc.Bacc("TRN2", target_bir_lowering=False, debug=False)

    xT = nc.dram_tensor("xT", [D, T], BF16, kind="ExternalInput")
    wq = nc.dram_tensor("wq", [D, E], BF16, kind="ExternalInput")
    wk = nc.dram_tensor("wk", [D, E], BF16, kind="ExternalInput")
    wv = nc.dram_tensor("wv", [D, E], BF16, kind="ExternalInput")
    wo = nc.dram_tensor("wo", [E, D], BF16, kind="ExternalInput")
    bq_d = nc.dram_tensor("bq", [E, 1], F32, kind="ExternalInput")
    bk_d = nc.dram_tensor("bk", [E, 1], F32, kind="ExternalInput")
    bvb_d = nc.dram_tensor("bvb", [128, E], F32, kind="ExternalInput")
    onesc_d = nc.dram_tensor("onesc", [128, 4], F32, kind="ExternalInput")
    onesr_d = nc.dram_tensor("onesr", [33, HD], BF16, kind="ExternalInput")
    outT = nc.dram_tensor("outT", [D, T], BF16, kind="ExternalOutput")

    with tile.TileContext(nc) as tc, nc.allow_low_precision(reason="bf16 attn"):
        with (
            tc.tile_pool(name="persist", bufs=1) as pp,
            tc.tile_pool(name="xw", bufs=1) as xw,
            tc.tile_pool(name="work", bufs=6) as wk_pool,
            tc.tile_pool(name="ostage", bufs=2) as op_pool,
            tc.tile_pool(name="small", bufs=2) as sm,
            tc.tile_pool(name="psum", bufs=2, space="PSUM") as ps,
        ):
            # ---- big SBUF tiles: all 8 d-chunks in one tile (merged DMAs)
            xa = xw.tile([128, DM, T], BF16, tag="xa", name="xa")
            wqa = xw.tile([128, DM, E], BF16, tag="wqa", name="wqa")
            wka = xw.tile([128, DM, E], BF16, tag="wka", name="wka")
            wva = xw.tile([128, DM, E], BF16, tag="wva", name="wva")
            woa = pp.tile([128, 2, D], BF16, tag="woa", name="woa")

            wq_r = wq[:].rearrange("(c p) e -> p c e", p=128)
            wk_r = wk[:].rearrange("(c p) e -> p c e", p=128)
            wv_r = wv[:].rearrange("(c p) e -> p c e", p=128)
            wo_r = wo[:].rearrange("(d p) e -> p d e", p=128)
            xT_r = xT[:].rearrange("(c p) t -> p c t", p=128)

            # wq first (q chain is first consumer; split so the first half
            # lands sooner), x block 0 per-chunk spread across engine queues
            # so descriptor processing runs on several DMA rings at once.
            nc.sync.dma_start(out=wqa[:, 0, 0:128], in_=wq_r[:, 0, 0:128])
            nc.sync.dma_start(out=wqa[:, 1:DM, 0:128], in_=wq_r[:, 1:DM, 0:128])
            nc.sync.dma_start(out=wqa[:, :, 128:E], in_=wq_r[:, :, 128:E])
            qs = [nc.scalar, nc.gpsimd]
            for c in range(DM):
                qs[c % 2].dma_start(out=xa[:, c, 0:TQ],
                                    in_=xT[c * 128:(c + 1) * 128, 0:TQ])
            nc.sync.dma_start(out=wka[:], in_=wk_r)
            bvb = pp.tile([128, E], F32, tag="bvb")
            nc.sync.dma_start(out=bvb[:], in_=bvb_d[:, :])
            bq_sb, bk_sb = [], []
            for e2 in range(2):
                t_ = pp.tile([128, 1], F32, tag=f"bq{e2}")
                nc.sync.dma_start(out=t_[:], in_=bq_d[e2 * 128:(e2 + 1) * 128, :])
                bq_sb.append(t_)
                t_ = pp.tile([128, 1], F32, tag=f"bk{e2}")
                nc.sync.dma_start(out=t_[:], in_=bk_d[e2 * 128:(e2 + 1) * 128, :])
                bk_sb.append(t_)
            onesc = pp.tile([128, 4], F32, tag="onesc")
            nc.sync.dma_start(out=onesc[:], in_=onesc_d[:, :])
            onesr = pp.tile([33, HD], BF16, tag="onesr")
            nc.sync.dma_start(out=onesr[:], in_=onesr_d[:, :])
            nc.sync.dma_start(out=wva[:], in_=wv_r)
            nc.sync.dma_start(out=xa[:, :, TQ:2 * TQ], in_=xT_r[:, :, TQ:2 * TQ])
            nc.sync.dma_start(out=xa[:, :, 2 * TQ:3 * TQ], in_=xT_r[:, :, 2 * TQ:3 * TQ])
            nc.sync.dma_start(out=woa[:], in_=wo_r)
            nc.sync.dma_start(out=xa[:, :, 3 * TQ:4 * TQ], in_=xT_r[:, :, 3 * TQ:4 * TQ])

            dn = pp.tile([33, TQ], F32, tag="dn")
            nc.vector.memset(dn[:], 1.0)
            qT_sb = [pp.tile([128, T], BF16, tag=f"qT{i}", name=f"qT{i}") for i in range(2)]
            kT_sb = [pp.tile([128, T], BF16, tag=f"kT{i}", name=f"kT{i}") for i in range(2)]
            v_sb = [pp.tile([128, 4, HD + 1], BF16, tag=f"v{t}", name=f"v{t}")
                    for t in range(NTKT)]
            yT_sb = [pp.tile([128, T], BF16, tag=f"yT{i}", name=f"yT{i}") for i in range(2)]

            def project_qk(tq):
                for (w_t, b_sb, dst) in ((wqa, bq_sb, qT_sb), (wka, bk_sb, kT_sb)):
                    for e2 in range(2):
                        pt = ps.tile([128, 1024], F32, tag="S",
                                     name=f"ppqk_{tq}_{e2}")
                        for c in range(DM):
                            nc.tensor.matmul(
                                pt[:, 0:TQ],
                                w_t[:, c, e2 * 128:(e2 + 1) * 128],
                                xa[:, c, tq * TQ:(tq + 1) * TQ],
                                start=(c == 0), stop=(c == DM - 1))
                        nc.vector.tensor_scalar_add(
                            out=dst[e2][:, tq * TQ:(tq + 1) * TQ],
                            in0=pt[:, 0:TQ], scalar1=b_sb[e2][:])

            def project_v(t):
                pt = ps.tile([128, E], F32, tag="y", name=f"ppv_{t}")
                for c in range(DM):
                    nc.tensor.matmul(
                        pt[:],
                        xa[:, c, t * 128:(t + 1) * 128],
                        wva[:, c, :],
                        start=(c == 0), stop=(c == DM - 1))
                nc.vector.tensor_add(
                    out=v_sb[t][:, :, 0:HD],
                    in0=pt[:].rearrange("p (h d) -> p h d", h=4),
                    in1=bvb[:].rearrange("p (h d) -> p h d", h=4))
                nc.vector.tensor_copy(
                    out=v_sb[t][:, :, HD:HD + 1],
                    in_=onesc[:].rearrange("p (h o) -> p h o", o=1))

            def out_proj_block(tq_o, final=False):
                for e8 in range(8):
                    pt = ps.tile([128, TQ], F32, tag="b", name=f"poc_{tq_o}_{e8}")
                    for d2 in range(2):
                        nc.tensor.matmul(
                            pt[:, 0:TQ],
                            woa[:, d2, e8 * 128:(e8 + 1) * 128],
                            yT_sb[d2][:, tq_o * TQ:(tq_o + 1) * TQ],
                            start=(d2 == 0), stop=(d2 == 1))
                    ot = op_pool.tile([128, TQ], BF16, tag="ostage",
                                      name=f"oto_{tq_o}_{e8}")
                    # in the final block the scalar engine is done with exp
                    # work, so alternate the staging copies onto it to halve
                    # the psum-recycle chain in the drain.
                    if final and e8 % 2 == 1:
                        nc.scalar.copy(out=ot[:], in_=pt[:])
                    else:
                        nc.vector.tensor_copy(out=ot[:], in_=pt[:])
                    nc.sync.dma_start(
                        out=outT[e8 * 128:(e8 + 1) * 128,
                                 tq_o * TQ:(tq_o + 1) * TQ],
                        in_=ot[:])

            def attention(tq):
                ntk = 4 * (tq + 1)
                for pr in range(2):
                    kt = kT_sb[pr]
                    qt = qT_sb[pr]
                    py_a = ps.tile([HD + 1, TQ], F32, tag="y", name=f"pya_{tq}_{pr}")
                    py_b = ps.tile([HD + 1, TQ], F32, tag="y", name=f"pyb_{tq}_{pr}")

                    def s_stage(tk):
                        # diag tiles only need columns >= 128*o (o = tk - 4*tq)
                        o = tk - 4 * tq
                        c0 = 128 * o if o > 0 else 0
                        n = TQ - c0
                        ps_s = ps.tile([128, 1024], F32, tag="S",
                                       name=f"ps_s_{tq}_{pr}_{tk}")
                        q0 = tq * TQ + c0
                        nc.tensor.matmul(
                            ps_s[:, c0:TQ],
                            kt[0:64, tk * 128:(tk + 1) * 128],
                            qt[0:64, q0:(tq + 1) * TQ],
                            start=True, stop=True)
                        nc.tensor.matmul(
                            ps_s[:, TQ + c0:2 * TQ],
                            kt[64:128, tk * 128:(tk + 1) * 128],
                            qt[64:128, q0:(tq + 1) * TQ],
                            start=True, stop=True)
                        es = wk_pool.tile([128, 1024], BF16, tag="expS",
                                          name=f"es_{tq}_{pr}_{tk}")
                        if c0 == 0:
                            nc.scalar.activation(es[:], ps_s[:], AF.Exp, scale=0.125)
                        elif c0 <= 256 and not sim_safe:
                            # one contiguous op; the junk span is never read
                            nc.scalar.activation(
                                es[:, c0:2 * TQ], ps_s[:, c0:2 * TQ],
                                AF.Exp, scale=0.125)
                        else:
                            for j in range(2):
                                nc.scalar.activation(
                                    es[:, j * TQ + c0:(j + 1) * TQ],
                                    ps_s[:, j * TQ + c0:(j + 1) * TQ],
                                    AF.Exp, scale=0.125)
                        if o >= 0:
                            em = wk_pool.tile([128, 1024], BF16, tag="expS",
                                              name=f"em_{tq}_{pr}_{tk}")
                            for j in range(2):
                                nc.gpsimd.affine_select(
                                    out=em[:, j * TQ + c0:(j + 1) * TQ],
                                    in_=es[:, j * TQ + c0:(j + 1) * TQ],
                                    compare_op=mybir.AluOpType.is_ge,
                                    fill=0.0,
                                    base=0,
                                    pattern=[[1, n]],
                                    channel_multiplier=-1)
                            es = em
                        return es, c0

                    # two-tile lookahead: exp/select for tile tk runs while
                    # the PE streams the next s-stages and earlier y-stages.
                    # Diag tiles (which add an affine_select hop) are
                    # interleaved with non-diag tiles so their longer
                    # act+pool latency chain gets two full tiles of cover.
                    nd = list(range(4 * tq))
                    dg = list(range(4 * tq, ntk))
                    order = []
                    while dg or nd:
                        if dg:
                            order.append(dg.pop(0))
                        if nd:
                            order.append(nd.pop(0))
                    first, last = order[0], order[-1]

                    def y_stage2(tk, es, c0):
                        nc.tensor.matmul(
                            py_a[:, c0:TQ], v_sb[tk][:, 2 * pr, :],
                            es[:, c0:TQ],
                            start=(tk == first), stop=(tk == last))
                        nc.tensor.matmul(
                            py_b[:, c0:TQ], v_sb[tk][:, 2 * pr + 1, :],
                            es[:, TQ + c0:2 * TQ],
                            start=(tk == first), stop=(tk == last))

                    done = {}
                    done[order[0]] = s_stage(order[0])
                    if ntk > 1:
                        done[order[1]] = s_stage(order[1])
                    for idx in range(2, ntk):
                        tk = order[idx]
                        done[tk] = s_stage(tk)
                        tk2 = order[idx - 2]
                        y_stage2(tk2, *done.pop(tk2))
                    if ntk > 1:
                        tk2 = order[ntk - 2]
                        y_stage2(tk2, *done.pop(tk2))
                    tk2 = order[ntk - 1]
                    y_stage2(tk2, *done.pop(tk2))

                    nc.vector.tensor_copy(out=dn[0:1, :], in_=py_a[HD:HD + 1, :])
                    nc.vector.tensor_copy(out=dn[32:33, :], in_=py_b[HD:HD + 1, :])
                    rc32 = sm.tile([33, TQ], F32, tag="rc32")
                    nc.vector.reciprocal_approx_fast(out=rc32[:, :], in_=dn[:, :])
                    rc = sm.tile([33, TQ], BF16, tag="rc")
                    nc.vector.tensor_copy(out=rc[:, :], in_=rc32[:, :])
                    for (i, py) in ((0, py_a), (1, py_b)):
                        pb = ps.tile([HD, TQ], F32, tag="b", name=f"pb_{tq}_{pr}_{i}")
                        nc.tensor.matmul(pb[:], onesr[32 * i:32 * i + 1, :],
                                         rc[32 * i:32 * i + 1, :],
                                         start=True, stop=True)
                        bc = sm.tile([HD, TQ], F32, tag="bc")
                        nc.vector.tensor_copy(out=bc[:], in_=pb[:])
                        row0 = i * 64
                        nc.vector.tensor_mul(
                            out=yT_sb[pr][row0:row0 + 64, tq * TQ:(tq + 1) * TQ],
                            in0=py[0:HD, :], in1=bc[:])

            # ---- interleaved emission: per tq block, project then attend,
            # then flush the previous block's output projection.
            for tq in range(NTQ):
                project_qk(tq)
                for t in range(4 * tq, 4 * tq + 4):
                    project_v(t)
                attention(tq)
                if tq > 0:
                    out_proj_block(tq - 1)
            out_proj_block(NTQ - 1, final=True)

    nc.compile()
    return nc


def _get_nc():
    if 'nc' not in _CACHE:
        _CACHE['nc'] = _build()
    return _CACHE['nc']


def _make_in_maps(x, Wq, bq, Wk, bk, Wv, bv, Wo, bo):
    import ml_dtypes
    bf16 = ml_dtypes.bfloat16
    x = np.asarray(x, dtype=np.float32)
    Wq = np.asarray(Wq, dtype=np.float32)
    Wk = np.asarray(Wk, dtype=np.float32)
    Wv = np.asarray(Wv, dtype=np.float32)
    Wo = np.asarray(Wo, dtype=np.float32)
    bq = np.asarray(bq, dtype=np.float32)
    bk = np.asarray(bk, dtype=np.float32)
    bv = np.asarray(bv, dtype=np.float32)

    onesc = np.ones((128, 4), dtype=np.float32)
    onesr = np.ones((33, HD), dtype=bf16)

    in_maps = []
    for c in range(NCORES):
        b, g = divmod(c, 4)
        hs = slice(g * E, (g + 1) * E)
        in_maps.append({
            "xT": np.ascontiguousarray(x[b].T.astype(bf16)),
            "wq": np.ascontiguousarray(Wq[hs].T.astype(bf16)),
            "wk": np.ascontiguousarray(Wk[hs].T.astype(bf16)),
            "wv": np.ascontiguousarray(Wv[hs].T.astype(bf16)),
            "wo": np.ascontiguousarray(Wo[:, hs].T.astype(bf16)),
            "bq": np.ascontiguousarray(bq[hs].reshape(E, 1)),
            "bk": np.ascontiguousarray(bk[hs].reshape(E, 1)),
            "bvb": np.broadcast_to(bv[hs], (128, E)).copy(),
            "onesc": onesc,
            "onesr": onesr,
        })
    return in_maps


def kernel(x, Wq, bq, Wk, bk, Wv, bv, Wo, bo, _run_kwargs=None):
    nc = _get_nc()
    in_maps = _make_in_maps(x, Wq, bq, Wk, bk, Wv, bv, Wo, bo)
    last_err = None
    for _attempt in range(3):
        try:
            res = run_bass_kernel_spmd(nc, in_maps, core_ids=list(range(NCORES)),
                                       **(_run_kwargs or {}))
            break
        except Exception as e:  # transient NRT/device hiccups: retry
            last_err = e
            import time as _time
            _time.sleep(2.0)
    else:
        raise last_err
    bo = np.asarray(bo, dtype=np.float32)
    out = np.empty((B, T, D), dtype=np.float32)
    for b in range(B):
        acc = res.results[4 * b]["outT"].astype(np.float32)
        for g in range(1, 4):
            acc += res.results[4 * b + g]["outT"].astype(np.float32)
        out[b] = acc.T + bo
    if _run_kwargs:
        _CACHE['last_results'] = res
    return out


# revision 39
# speedup vs baseline: 1.0327x; 1.0327x over previous
"""Causal self-attention (B=2, T=2048, D=1024, H=16) on 8 TRN2 NeuronCores.

Sharding: data-parallel over batch (cores 0-3 -> batch 0, cores 4-7 -> batch 1),
tensor-parallel over heads (4 heads / 256 output dims per core). Each core
computes q/k/v projections for its heads, causal flash-style attention, and a
partial output projection (contraction over its 256 dims of Wo). The host sums
the 4 partials per batch and adds bo.

All matmul operands are bf16 (fp32 PSUM accumulation). Inputs/outputs move as
bf16, halving HBM traffic vs fp32. Weights and trailing x blocks load via
single merged DMAs to keep the sync engine's DMA-issue serialization off the
critical path; the attention inner loop runs exp two k-tiles ahead of the
accumulating value matmuls so PE never waits on the scalar engine.
"""
import sys

sys.path.insert(0, '/opt/trn_rl_repo')

import numpy as np

import concourse.bass as bass  # noqa: F401  (import keeps bass registered)
import concourse.mybir as mybir
import concourse.tile as tile
from concourse import bacc
from concourse.bass_utils import run_bass_kernel_spmd

F32 = mybir.dt.float32
BF16 = mybir.dt.bfloat16
AF = mybir.ActivationFunctionType

B, T, D, H, HD = 2, 2048, 1024, 16, 64
NCORES = 8
E = 256          # output dims per core (4 heads x 64)
DM = 8           # d_model chunks of 128
TQ = 512
NTQ = T // TQ    # 4
TKT = 128
NTKT = T // TKT  # 16

_CACHE = {}


def _build(sim_safe=False):
    nc = bacc.Bacc("TRN2", target_bir_lowering=False, debug=False)

    xT = nc.dram_tensor("xT", [D, T], BF16, kind="ExternalInput")
    wq = nc.dram_tensor("wq", [D, E], BF16, kind="ExternalInput")
    wk = nc.dram_tensor("wk", [D, E], BF16, kind="ExternalInput")
    wv = nc.dram_tensor("wv", [D, E], BF16, kind="ExternalInput")
    wo = nc.dram_tensor("wo", [E, D], BF16, kind="ExternalInput")
    bq_d = nc.dram_tensor("bq", [E, 1], F32, kind="ExternalInput")
    bk_d = nc.dram_tensor("bk", [E, 1], F32, kind="ExternalInput")
    bvb_d = nc.dram_tensor("bvb", [128, E], F32, kind="ExternalInput")
    onesc_d = nc.dram_tensor("onesc", [128, 4], F32, kind="ExternalInput")
    onesr_d = nc.dram_tensor("onesr", [33, HD], BF16, kind="ExternalInput")
    outT = nc.dram_tensor("outT", [D, T], BF16, kind="ExternalOutput")

    with tile.TileContext(nc) as tc, nc.allow_low_precision(reason="bf16 attn"):
        with (
            tc.tile_pool(name="persist", bufs=1) as pp,
            tc.tile_pool(name="xw", bufs=1) as xw,
            tc.tile_pool(name="work", bufs=6) as wk_pool,
            tc.tile_pool(name="ostage", bufs=2) as op_pool,
            tc.tile_pool(name="small", bufs=2) as sm,
            tc.tile_pool(name="psum", bufs=2, space="PSUM") as ps,
        ):
            # ---- big SBUF tiles: all 8 d-chunks in one tile (merged DMAs)
            xa = xw.tile([128, DM, T], BF16, tag="xa", name="xa")
            wqa = xw.tile([128, DM, E], BF16, tag="wqa", name="wqa")
            wka = xw.tile([128, DM, E], BF16, tag="wka", name="wka")
            wva = xw.tile([128, DM, E], BF16, tag="wva", name="wva")
            woa = pp.tile([128, 2, D], BF16, tag="woa", name="woa")

            wq_r = wq[:].rearrange("(c p) e -> p c e", p=128)
            wk_r = wk[:].rearrange("(c p) e -> p c e", p=128)
            wv_r = wv[:].rearrange("(c p) e -> p c e", p=128)
            wo_r = wo[:].rearrange("(d p) e -> p d e", p=128)
            xT_r = xT[:].rearrange("(c p) t -> p c t", p=128)

            # wq first (q chain is first consumer; split so the first half
            # lands sooner), x block 0 per-chunk spread across engine queues
            # so descriptor processing runs on several DMA rings at once.
            nc.sync.dma_start(out=wqa[:, 0, 0:128], in_=wq_r[:, 0, 0:128])
            nc.sync.dma_start(out=wqa[:, 1:DM, 0:128], in_=wq_r[:, 1:DM, 0:128])
            nc.sync.dma_start(out=wqa[:, :, 128:E], in_=wq_r[:, :, 128:E])
            qs = [nc.scalar, nc.gpsimd]
            for c in range(DM):
                qs[c % 2].dma_start(out=xa[:, c, 0:TQ],
                                    in_=xT[c * 128:(c + 1) * 128, 0:TQ])
            nc.sync.dma_start(out=wka[:], in_=wk_r)
            bvb = pp.tile([128, E], F32, tag="bvb")
            nc.sync.dma_start(out=bvb[:], in_=bvb_d[:, :])
            bq_sb, bk_sb = [], []
            for e2 in range(2):
                t_ = pp.tile([128, 1], F32, tag=f"bq{e2}")
                nc.sync.dma_start(out=t_[:], in_=bq_d[e2 * 128:(e2 + 1) * 128, :])
                bq_sb.append(t_)
                t_ = pp.tile([128, 1], F32, tag=f"bk{e2}")
                nc.sync.dma_start(out=t_[:], in_=bk_d[e2 * 128:(e2 + 1) * 128, :])
                bk_sb.append(t_)
            onesc = pp.tile([128, 4], F32, tag="onesc")
            nc.sync.dma_start(out=onesc[:], in_=onesc_d[:, :])
            onesr = pp.tile([33, HD], BF16, tag="onesr")
            nc.sync.dma_start(out=onesr[:], in_=onesr_d[:, :])
            nc.sync.dma_start(out=wva[:], in_=wv_r)
            nc.sync.dma_start(out=xa[:, :, TQ:2 * TQ], in_=xT_r[:, :, TQ:2 * TQ])
            nc.sync.dma_start(out=xa[:, :, 2 * TQ:3 * TQ], in_=xT_r[:, :, 2 * TQ:3 * TQ])
            nc.sync.dma_start(out=woa[:], in_=wo_r)
            nc.sync.dma_start(out=xa[:, :, 3 * TQ:4 * TQ], in_=xT_r[:, :, 3 * TQ:4 * TQ])

            dn = pp.tile([33, TQ], F32, tag="dn")
            nc.vector.memset(dn[:], 1.0)
            qT_sb = [pp.tile([128, T], BF16, tag=f"qT{i}", name=f"qT{i}") for i in range(2)]
            kT_sb = [pp.tile([128, T], BF16, tag=f"kT{i}", name=f"kT{i}") for i in range(2)]
            v_sb = [pp.tile([128, 4, HD + 1], BF16, tag=f"v{t}", name=f"v{t}")
                    for t in range(NTKT)]
            yT_sb = [pp.tile([128, T], BF16, tag=f"yT{i}", name=f"yT{i}") for i in range(2)]

            def project_qk(tq):
                for (w_t, b_sb, dst) in ((wqa, bq_sb, qT_sb), (wka, bk_sb, kT_sb)):
                    for e2 in range(2):
                        pt = ps.tile([128, 1024], F32, tag="S",
                                     name=f"ppqk_{tq}_{e2}")
                        for c in range(DM):
                            nc.tensor.matmul(
                                pt[:, 0:TQ],
                                w_t[:, c, e2 * 128:(e2 + 1) * 128],
                                xa[:, c, tq * TQ:(tq + 1) * TQ],
                                start=(c == 0), stop=(c == DM - 1))
                        nc.vector.tensor_scalar_add(
                            out=dst[e2][:, tq * TQ:(tq + 1) * TQ],
                            in0=pt[:, 0:TQ], scalar1=b_sb[e2][:])

            def project_v(t):
                pt = ps.tile([128, E], F32, tag="y", name=f"ppv_{t}")
                for c in range(DM):
                    nc.tensor.matmul(
                        pt[:],
                        xa[:, c, t * 128:(t + 1) * 128],
                        wva[:, c, :],
                        start=(c == 0), stop=(c == DM - 1))
                nc.vector.tensor_add(
                    out=v_sb[t][:, :, 0:HD],
                    in0=pt[:].rearrange("p (h d) -> p h d", h=4),
                    in1=bvb[:].rearrange("p (h d) -> p h d", h=4))
                nc.vector.tensor_copy(
                    out=v_sb[t][:, :, HD:HD + 1],
                    in_=onesc[:].rearrange("p (h o) -> p h o", o=1))

            def out_proj_block(tq_o, final=False):
                for e8 in range(8):
                    pt = ps.tile([128, TQ], F32, tag="b", name=f"poc_{tq_o}_{e8}")
                    for d2 in range(2):
                        nc.tensor.matmul(
                            pt[:, 0:TQ],
                            woa[:, d2, e8 * 128:(e8 + 1) * 128],
                            yT_sb[d2][:, tq_o * TQ:(tq_o + 1) * TQ],
                            start=(d2 == 0), stop=(d2 == 1))
                    ot = op_pool.tile([128, TQ], BF16, tag="ostage",
                                      name=f"oto_{tq_o}_{e8}")
                    # in the final block the scalar engine is done with exp
                    # work, so alternate the staging copies onto it to halve
                    # the psum-recycle chain in the drain.
                    if final and e8 % 2 == 1:
                        nc.scalar.copy(out=ot[:], in_=pt[:])
                    else:
                        nc.vector.tensor_copy(out=ot[:], in_=pt[:])
                    nc.sync.dma_start(
                        out=outT[e8 * 128:(e8 + 1) * 128,
                                 tq_o * TQ:(tq_o + 1) * TQ],
                        in_=ot[:])

            def attention(tq):
                ntk = 4 * (tq + 1)
                for pr in range(2):
                    kt = kT_sb[pr]
                    qt = qT_sb[pr]
                    py_a = ps.tile([HD + 1, TQ], F32, tag="y", name=f"pya_{tq}_{pr}")
                    py_b = ps.tile([HD + 1, TQ], F32, tag="y", name=f"pyb_{tq}_{pr}")

                    def s_stage(tk):
                        # diag tiles only need columns >= 128*o (o = tk - 4*tq)
                        o = tk - 4 * tq
                        c0 = 128 * o if o > 0 else 0
                        n = TQ - c0
                        ps_s = ps.tile([128, 1024], F32, tag="S",
                                       name=f"ps_s_{tq}_{pr}_{tk}")
                        q0 = tq * TQ + c0
                        nc.tensor.matmul(
                            ps_s[:, c0:TQ],
                            kt[0:64, tk * 128:(tk + 1) * 128],
                            qt[0:64, q0:(tq + 1) * TQ],
                            start=True, stop=True)
                        nc.tensor.matmul(
                            ps_s[:, TQ + c0:2 * TQ],
                            kt[64:128, tk * 128:(tk + 1) * 128],
                            qt[64:128, q0:(tq + 1) * TQ],
                            start=True, stop=True)
                        es = wk_pool.tile([128, 1024], BF16, tag="expS",
                                          name=f"es_{tq}_{pr}_{tk}")
                        if c0 == 0:
                            nc.scalar.activation(es[:], ps_s[:], AF.Exp, scale=0.125)
                        elif c0 <= 256 and not sim_safe:
                            # one contiguous op; the junk span is never read
                            nc.scalar.activation(
                                es[:, c0:2 * TQ], ps_s[:, c0:2 * TQ],
                                AF.Exp, scale=0.125)
                        else:
                            for j in range(2):
                                nc.scalar.activation(
                                    es[:, j * TQ + c0:(j + 1) * TQ],
                                    ps_s[:, j * TQ + c0:(j + 1) * TQ],
                                    AF.Exp, scale=0.125)
                        if o >= 0:
                            em = wk_pool.tile([128, 1024], BF16, tag="expS",
                                              name=f"em_{tq}_{pr}_{tk}")
                            for j in range(2):
                                nc.gpsimd.affine_select(
                                    out=em[:, j * TQ + c0:(j + 1) * TQ],
                                    in_=es[:, j * TQ + c0:(j + 1) * TQ],
                                    compare_op=mybir.AluOpType.is_ge,
                                    fill=0.0,
                                    base=0,
                                    pattern=[[1, n]],
                                    channel_multiplier=-1)
                            es = em
                        return es, c0

                    # two-tile lookahead: exp/select for tile tk runs while
                    # the PE streams the next s-stages and earlier y-stages.
                    # Diag tiles (which add an affine_select hop) are
                    # interleaved with non-diag tiles so their longer
                    # act+pool latency chain gets two full tiles of cover.
                    nd = list(range(4 * tq))
                    dg = list(range(4 * tq, ntk))
                    order = []
                    while dg or nd:
                        if dg:
                            order.append(dg.pop(0))
                        if nd:
                            order.append(nd.pop(0))
                    first, last = order[0], order[-1]

                    def y_stage2(tk, es, c0):
                        nc.tensor.matmul(
                            py_a[:, c0:TQ], v_sb[tk][:, 2 * pr, :],
                            es[:, c0:TQ],
                            start=(tk == first), stop=(tk == last))
                        nc.tensor.matmul(
                            py_b[:, c0:TQ], v_sb[tk][:, 2 * pr + 1, :],
                            es[:, TQ + c0:2 * TQ],
                            start=(tk == first), stop=(tk == last))

                    done = {}
                    done[order[0]] = s_stage(order[0])
                    if ntk > 1:
                        done[order[1]] = s_stage(order[1])
                    for idx in range(2, ntk):
                        tk = order[idx]
                        done[tk] = s_stage(tk)
                        tk2 = order[idx - 2]
                        y_stage2(tk2, *done.pop(tk2))
                    if ntk > 1:
                        tk2 = order[ntk - 2]
                        y_stage2(tk2, *done.pop(tk2))
                    tk2 = order[ntk - 1]
                    y_stage2(tk2, *done.pop(tk2))

                    nc.vector.tensor_copy(out=dn[0:1, :], in_=py_a[HD:HD + 1, :])
                    nc.vector.tensor_copy(out=dn[32:33, :], in_=py_b[HD:HD + 1, :])
                    rc32 = sm.tile([33, TQ], F32, tag="rc32")
                    nc.vector.reciprocal_approx_fast(out=rc32[:, :], in_=dn[:, :])
                    rc = sm.tile([33, TQ], BF16, tag="rc")
                    nc.vector.tensor_copy(out=rc[:, :], in_=rc32[:, :])
                    for (i, py) in ((0, py_a), (1, py_b)):
                        pb = ps.tile([HD, TQ], F32, tag="b", name=f"pb_{tq}_{pr}_{i}")
                        nc.tensor.matmul(pb[:], onesr[32 * i:32 * i + 1, :],
                                         rc[32 * i:32 * i + 1, :],
                                         start=True, stop=True)
                        bc = sm.tile([HD, TQ], F32, tag="bc")
                        nc.vector.tensor_copy(out=bc[:], in_=pb[:])
                        row0 = i * 64
                        nc.vector.tensor_mul(
                            out=yT_sb[pr][row0:row0 + 64, tq * TQ:(tq + 1) * TQ],
                            in0=py[0:HD, :], in1=bc[:])

            # ---- interleaved emission: per tq block, project then attend,
            # then flush the previous block's output projection.
            # oproj for the previous block is emitted AFTER the next block's
            # projections: the q/k bias adds then sit ahead of the 8 oproj
            # staging copies in the DVE queue, so the first s-stages of
            # attention(tq) get their q/k tiles without stalling, while the
            # oproj copies (latency-tolerant) drain during attention.
            for tq in range(NTQ):
                project_qk(tq)
                for t in range(4 * tq, 4 * tq + 4):
                    project_v(t)
                if tq > 0:
                    out_proj_block(tq - 1)
                attention(tq)
            out_proj_block(NTQ - 1, final=True)

    nc.compile()
    return nc


def _get_nc():
    if 'nc' not in _CACHE:
        _CACHE['nc'] = _build()
    return _CACHE['nc']


def _make_in_maps(x, Wq, bq, Wk, bk, Wv, bv, Wo, bo):
    import ml_dtypes
    bf16 = ml_dtypes.bfloat16
    x = np.asarray(x, dtype=np.float32)
    Wq = np.asarray(Wq, dtype=np.float32)
    Wk = np.asarray(Wk, dtype=np.float32)
    Wv = np.asarray(Wv, dtype=np.float32)
    Wo = np.asarray(Wo, dtype=np.float32)
    bq = np.asarray(bq, dtype=np.float32)
    bk = np.asarray(bk, dtype=np.float32)
    bv = np.asarray(bv, dtype=np.float32)

    onesc = np.ones((128, 4), dtype=np.float32)
    onesr = np.ones((33, HD), dtype=bf16)

    in_maps = []
    for c in range(NCORES):
        b, g = divmod(c, 4)
        hs = slice(g * E, (g + 1) * E)
        in_maps.append({
            "xT": np.ascontiguousarray(x[b].T.astype(bf16)),
            "wq": np.ascontiguousarray(Wq[hs].T.astype(bf16)),
            "wk": np.ascontiguousarray(Wk[hs].T.astype(bf16)),
            "wv": np.ascontiguousarray(Wv[hs].T.astype(bf16)),
            "wo": np.ascontiguousarray(Wo[:, hs].T.astype(bf16)),
            "bq": np.ascontiguousarray(bq[hs].reshape(E, 1)),
            "bk": np.ascontiguousarray(bk[hs].reshape(E, 1)),
            "bvb": np.broadcast_to(bv[hs], (128, E)).copy(),
            "onesc": onesc,
            "onesr": onesr,
        })
    return in_maps


def kernel(x, Wq, bq, Wk, bk, Wv, bv, Wo, bo, _run_kwargs=None):
    nc = _get_nc()
    in_maps = _make_in_maps(x, Wq, bq, Wk, bk, Wv, bv, Wo, bo)
    last_err = None
    for _attempt in range(3):
        try:
            res = run_bass_kernel_spmd(nc, in_maps, core_ids=list(range(NCORES)),
                                       **(_run_kwargs or {}))
            break
        except Exception as e:  # transient NRT/device hiccups: retry
            last_err = e
            import time as _time
            _time.sleep(2.0)
    else:
        raise last_err
    bo = np.asarray(bo, dtype=np.float32)
    out = np.empty((B, T, D), dtype=np.float32)
    for b in range(B):
        acc = res.results[4 * b]["outT"].astype(np.float32)
        for g in range(1, 4):
            acc += res.results[4 * b + g]["outT"].astype(np.float32)
        out[b] = acc.T + bo
    if _run_kwargs:
        _CACHE['last_results'] = res
    return out
